# revision 1
# baseline (speedup 1.0000x reference)
"""Expected Calibration Error kernel for Trainium2 (Bass/Tile), 8 NeuronCores, v3.

Problem: logits [1000000, 100] f32, labels [1000000] i64 ->
  (ece [1] f32, acc [1] f32)

v3 strategy:
  - R rows/partition per tile (32 or 62), T = 992 // R tiles/core. Each tile's
    logits slab ([128, R, 100] f32, 1.6-3.2 MB) is one contiguous HBM->SBUF
    DMA, alternating between the two HWDGE queues (sync/scalar).
  - Per-tile DVE: grouped reduce_max -> conf; is_equal(chosen, conf) -> acc;
    is_gt(conf, bounds) -> cumulative bin masks G [128, R, 16].
  - v tiles hold interleaved triples (conf, acc, 1) per row-slot; the ones
    lane is written once at kernel start (tiles are pre-allocated per t and
    reused across reps).
  - Per-tile matmul chunks (rhs free dim <= 512): psum_ck [3*S, S*16] +=
    v[:, a:b, :]^T @ G[:, a:b, :], accumulated across tiles and reps.
  - Host folds the per-core psum chunks (diagonal r-slot blocks), differences
    cumulative bounds into 15 bins, applies the ECE formula.
"""

import numpy as np

P = 128          # SBUF partitions
C = 100          # classes
NCORES = 8
NB = 15          # bin boundaries used on-device (bound 15 = conf>1.0 == 0 always)
ROWS_PART = 992  # rows per partition per core
ROWS_CORE = P * ROWS_PART      # 126976
N = 1_000_000

R = 62           # rows per partition per tile
T = ROWS_PART // R
NCHUNK = (R * NB + 511) // 512
S = R // NCHUNK  # row-slots per matmul chunk

_CACHE = {}


def _build_nc(reps=1, xbufs=4, do_vec=True, do_pe=True, dma_mode="alt",
              gt_engine="vector", eq_engine="vector"):
    import concourse.bass as bass
    import concourse.bacc as bacc
    import concourse.mybir as mybir
    import concourse.tile as tile

    f32 = mybir.dt.float32
    nc = bacc.Bacc()

    logits_d = nc.dram_tensor("logits", [ROWS_CORE, C], f32, kind="ExternalInput")
    chosen_d = nc.dram_tensor("chosen", [P, T * R], f32, kind="ExternalInput")
    bounds_d = nc.dram_tensor("bounds", [1, NB], f32, kind="ExternalInput")
    stats_d = nc.dram_tensor("stats", [3 * S, NCHUNK * S * NB], f32,
                             kind="ExternalOutput")

    # [T, 128, R*C] view of the row-major logits: partition p of tile t holds
    # rows (t*128 + p)*R ... + R.
    lx = logits_d[:].flatten().rearrange("(t p f) -> t p f", t=T, p=P, f=R * C)

    with tile.TileContext(nc) as tc:
        with (
            tc.tile_pool(name="singles", bufs=1) as singles,
            tc.tile_pool(name="xtiles", bufs=xbufs) as xtiles,
            tc.tile_pool(name="vals", bufs=1) as valsp,
            tc.tile_pool(name="gmask", bufs=4) as gmaskp,
            tc.tile_pool(name="psum", bufs=1, space="PSUM") as psump,
        ):
            bounds_sb = singles.tile([P, NB], f32)
            nc.sync.dma_start(
                out=bounds_sb[:],
                in_=bass.AP(tensor=bounds_d, offset=0, ap=[[0, P], [1, NB]]),
            )
            chosen_sb = singles.tile([P, T * R], f32)
            nc.sync.dma_start(out=chosen_sb[:], in_=chosen_d[:])
            # First-touch the DMA'd singles on DVE so in-loop ops never carry
            # a second sync-wait (walrus core_v3 allows 1 wait/instruction).
            touch = singles.tile([P, 2], f32)
            nc.vector.tensor_copy(out=touch[:, 0:1], in_=chosen_sb[:, 0:1])
            nc.vector.tensor_copy(out=touch[:, 1:2], in_=bounds_sb[:, 0:1])

            # One v tile per t, reused across reps; ones lane written once.
            vlist = []
            for t in range(T):
                v = valsp.tile([P, R, 3], f32, tag=f"v{t}", name=f"v{t}")
                nc.vector.memset(v[:, :, 2], 1.0)
                vlist.append(v)

            psums = [psump.tile([3 * S, S * NB], f32, tag=f"ps{ck}", name=f"ps{ck}")
                     for ck in range(NCHUNK)]

            bounds_b = bounds_sb[:].unsqueeze(1).broadcast_to([P, R, NB])
            geng = nc.vector if gt_engine == "vector" else nc.gpsimd
            eeng = nc.vector if eq_engine == "vector" else nc.gpsimd

            for rep in range(reps):
              for t in range(T):
                x = xtiles.tile([P, R, C], f32)
                src = lx[t].rearrange("p (r c) -> p r c", r=R)
                if dma_mode == "sp":
                    nc.sync.dma_start(out=x[:], in_=src)
                elif dma_mode == "alt":
                    eng = nc.sync if t % 2 == 0 else nc.scalar
                    eng.dma_start(out=x[:], in_=src)
                else:
                    raise ValueError(dma_mode)

                if not do_vec:
                    dummy = gmaskp.tile([P, 1], f32, tag="dummy", name="dummy")
                    nc.vector.tensor_copy(out=dummy[:], in_=x[:, 0:1, 0])
                    continue
                v = vlist[t]
                conf = v[:, :, 0]
                nc.vector.tensor_reduce(
                    out=conf, in_=x[:], axis=mybir.AxisListType.X,
                    op=mybir.AluOpType.max,
                )
                eeng.tensor_tensor(
                    out=v[:, :, 1],
                    in0=chosen_sb[:, t * R:(t + 1) * R],
                    in1=conf,
                    op=mybir.AluOpType.is_equal,
                )
                g = gmaskp.tile([P, R, NB], f32)
                geng.tensor_tensor(
                    out=g[:],
                    in0=conf.unsqueeze(2).broadcast_to([P, R, NB]),
                    in1=bounds_b,
                    op=mybir.AluOpType.is_gt,
                )
                if do_pe:
                    first = (rep == 0 and t == 0)
                    last = (rep == reps - 1 and t == T - 1)
                    for ck in range(NCHUNK):
                        a, b = ck * S, (ck + 1) * S
                        nc.tensor.matmul(
                            psums[ck][:],
                            v[:, a:b, :].rearrange("p r s -> p (r s)"),
                            g[:, a:b, :].rearrange("p r j -> p (r j)"),
                            start=first, stop=last,
                        )

            out_sb = singles.tile([3 * S, NCHUNK * S * NB], f32)
            if do_vec and do_pe:
                for ck in range(NCHUNK):
                    nc.vector.tensor_copy(
                        out=out_sb[:, ck * S * NB:(ck + 1) * S * NB],
                        in_=psums[ck][:],
                    )
            else:
                nc.vector.memset(out_sb[:], 0.0)
            nc.sync.dma_start(out=stats_d[:], in_=out_sb[:])

    nc.finalize()
    return nc


def _get_nc():
    if "nc" not in _CACHE:
        _CACHE["nc"] = _build_nc()
    return _CACHE["nc"]


def _prep_inputs(logits, labels):
    """Shard + host-side prep. Returns in_maps for run_bass_kernel_spmd."""
    logits = np.asarray(logits)
    labels = np.asarray(labels)
    assert logits.shape == (N, C) and logits.dtype == np.float32

    bounds = np.linspace(0.0, 1.0, 16, dtype=np.float32)[:NB]
    chosen = np.take_along_axis(
        logits, labels.reshape(-1, 1).astype(np.int64), axis=1
    ).reshape(-1)

    in_maps = []
    for c in range(NCORES):
        lo = c * ROWS_CORE
        hi = lo + ROWS_CORE
        if hi <= N:
            lg = logits[lo:hi]           # view, no copy
            ch = chosen[lo:hi]
        else:
            npad = hi - N
            lg = np.vstack([logits[lo:N], np.full((npad, C), -1.0, np.float32)])
            ch = np.concatenate([chosen[lo:N], np.zeros(npad, np.float32)])
        ch_t = np.ascontiguousarray(
            ch.reshape(T, P, R).transpose(1, 0, 2).reshape(P, T * R)
        )
        in_maps.append({"logits": lg, "chosen": ch_t, "bounds": bounds.reshape(1, NB)})
    return in_maps


def _finish(outs):
    """Fold per-core [3S, NCHUNK*S*NB] stats into (ece, acc)."""
    cum_conf = np.zeros(NB, np.float64)
    cum_acc = np.zeros(NB, np.float64)
    cum_cnt = np.zeros(NB, np.float64)
    r_idx = np.arange(S)
    for o in outs:
        o = np.asarray(o, np.float64)
        for ck in range(NCHUNK):
            blk = o[:, ck * S * NB:(ck + 1) * S * NB]
            blk = blk.reshape(S, 3, S, NB)       # [r][s][r2][j]
            cum_conf += blk[r_idx, 0, r_idx, :].sum(axis=0)
            cum_acc += blk[r_idx, 1, r_idx, :].sum(axis=0)
            cum_cnt += blk[r_idx, 2, r_idx, :].sum(axis=0)

    count = cum_cnt - np.append(cum_cnt[1:], 0.0)
    sconf = cum_conf - np.append(cum_conf[1:], 0.0)
    sacc = cum_acc - np.append(cum_acc[1:], 0.0)

    safe = count > 0
    denom = np.where(safe, count, 1.0)
    conf_in = sconf / denom
    acc_in = sacc / denom
    prop = count / float(N)
    ece = float(np.where(safe, np.abs(conf_in - acc_in) * prop, 0.0).sum() * 100.0)
    acc = float(np.where(safe, acc_in * prop, 0.0).sum() * 100.0)
    return (
        np.array([ece], np.float32),
        np.array([acc], np.float32),
    )


def _run(logits, labels, trace=False):
    from concourse.bass_utils import run_bass_kernel_spmd

    nc = _get_nc()
    in_maps = _prep_inputs(logits, labels)
    res = run_bass_kernel_spmd(
        nc, in_maps, core_ids=list(range(NCORES)), trace=trace,
    )
    outs = [r["stats"] for r in res.results]
    return _finish(outs), res


def kernel(logits, labels):
    out, _ = _run(logits, labels)
    return out



# revision 3
# speedup vs baseline: 1.0512x; 1.0512x over previous
"""Expected Calibration Error kernel for Trainium2 (Bass/Tile), 8 NeuronCores, v3.

Problem: logits [1000000, 100] f32, labels [1000000] i64 ->
  (ece [1] f32, acc [1] f32)

v3 strategy:
  - R rows/partition per tile (32 or 62), T = 992 // R tiles/core. Each tile's
    logits slab ([128, R, 100] f32, 1.6-3.2 MB) is one contiguous HBM->SBUF
    DMA, alternating between the two HWDGE queues (sync/scalar).
  - Per-tile DVE: grouped reduce_max -> conf; is_equal(chosen, conf) -> acc;
    is_gt(conf, bounds) -> cumulative bin masks G [128, R, 16].
  - v tiles hold interleaved triples (conf, acc, 1) per row-slot; the ones
    lane is written once at kernel start (tiles are pre-allocated per t and
    reused across reps).
  - Per-tile matmul chunks (rhs free dim <= 512): psum_ck [3*S, S*16] +=
    v[:, a:b, :]^T @ G[:, a:b, :], accumulated across tiles and reps.
  - Host folds the per-core psum chunks (diagonal r-slot blocks), differences
    cumulative bounds into 15 bins, applies the ECE formula.
"""

import numpy as np

P = 128          # SBUF partitions
C = 100          # classes
NCORES = 8
NB = 15          # bin boundaries used on-device (bound 15 = conf>1.0 == 0 always)
ROWS_PART = 980  # rows per partition per core (980*128*8 = 1,003,520 >= N;
                 # closest tileable value to N/8/128 = 976.56 -> 1.2% less
                 # per-core HBM traffic than the previous 992)
ROWS_CORE = P * ROWS_PART      # 125440
N = 1_000_000

R = 28           # rows per partition per tile (28*15 = 420 <= 512: NCHUNK=1)
T = ROWS_PART // R
NCHUNK = (R * NB + 511) // 512
S = R // NCHUNK  # row-slots per matmul chunk

_CACHE = {}


def _build_nc(reps=1, xbufs=6, do_vec=True, do_pe=True, dma_mode="alt",
              gt_engine="vector", eq_engine="vector"):
    import concourse.bass as bass
    import concourse.bacc as bacc
    import concourse.mybir as mybir
    import concourse.tile as tile

    f32 = mybir.dt.float32
    nc = bacc.Bacc()

    logits_d = nc.dram_tensor("logits", [ROWS_CORE, C], f32, kind="ExternalInput")
    chosen_d = nc.dram_tensor("chosen", [P, T * R], f32, kind="ExternalInput")
    bounds_d = nc.dram_tensor("bounds", [1, NB], f32, kind="ExternalInput")
    stats_d = nc.dram_tensor("stats", [3 * S, NCHUNK * S * NB], f32,
                             kind="ExternalOutput")

    # [T, 128, R*C] view of the row-major logits: partition p of tile t holds
    # rows (t*128 + p)*R ... + R.
    lx = logits_d[:].flatten().rearrange("(t p f) -> t p f", t=T, p=P, f=R * C)

    with tile.TileContext(nc) as tc:
        with (
            tc.tile_pool(name="singles", bufs=1) as singles,
            tc.tile_pool(name="xtiles", bufs=xbufs) as xtiles,
            tc.tile_pool(name="vals", bufs=1) as valsp,
            tc.tile_pool(name="gmask", bufs=4) as gmaskp,
            tc.tile_pool(name="psum", bufs=1, space="PSUM") as psump,
        ):
            bounds_sb = singles.tile([P, NB], f32)
            nc.sync.dma_start(
                out=bounds_sb[:],
                in_=bass.AP(tensor=bounds_d, offset=0, ap=[[0, P], [1, NB]]),
            )
            chosen_sb = singles.tile([P, T * R], f32)
            nc.sync.dma_start(out=chosen_sb[:], in_=chosen_d[:])
            # First-touch the DMA'd singles on DVE so in-loop ops never carry
            # a second sync-wait (walrus core_v3 allows 1 wait/instruction).
            touch = singles.tile([P, 2], f32)
            nc.vector.tensor_copy(out=touch[:, 0:1], in_=chosen_sb[:, 0:1])
            nc.vector.tensor_copy(out=touch[:, 1:2], in_=bounds_sb[:, 0:1])

            # One v tile per t, reused across reps; ones lane written once.
            vlist = []
            for t in range(T):
                v = valsp.tile([P, R, 3], f32, tag=f"v{t}", name=f"v{t}")
                nc.vector.memset(v[:, :, 2], 1.0)
                vlist.append(v)

            psums = [psump.tile([3 * S, S * NB], f32, tag=f"ps{ck}", name=f"ps{ck}")
                     for ck in range(NCHUNK)]

            bounds_b = bounds_sb[:].unsqueeze(1).broadcast_to([P, R, NB])
            geng = nc.vector if gt_engine == "vector" else nc.gpsimd
            eeng = nc.vector if eq_engine == "vector" else nc.gpsimd

            for rep in range(reps):
              for t in range(T):
                x = xtiles.tile([P, R, C], f32)
                src = lx[t].rearrange("p (r c) -> p r c", r=R)
                if dma_mode == "sp":
                    nc.sync.dma_start(out=x[:], in_=src)
                elif dma_mode == "alt":
                    eng = nc.sync if t % 2 == 0 else nc.scalar
                    eng.dma_start(out=x[:], in_=src)
                else:
                    raise ValueError(dma_mode)

                if not do_vec:
                    dummy = gmaskp.tile([P, 1], f32, tag="dummy", name="dummy")
                    nc.vector.tensor_copy(out=dummy[:], in_=x[:, 0:1, 0])
                    continue
                v = vlist[t]
                conf = v[:, :, 0]
                nc.vector.tensor_reduce(
                    out=conf, in_=x[:], axis=mybir.AxisListType.X,
                    op=mybir.AluOpType.max,
                )
                eeng.tensor_tensor(
                    out=v[:, :, 1],
                    in0=chosen_sb[:, t * R:(t + 1) * R],
                    in1=conf,
                    op=mybir.AluOpType.is_equal,
                )
                g = gmaskp.tile([P, R, NB], f32)
                geng.tensor_tensor(
                    out=g[:],
                    in0=conf.unsqueeze(2).broadcast_to([P, R, NB]),
                    in1=bounds_b,
                    op=mybir.AluOpType.is_gt,
                )
                if do_pe:
                    first = (rep == 0 and t == 0)
                    last = (rep == reps - 1 and t == T - 1)
                    for ck in range(NCHUNK):
                        a, b = ck * S, (ck + 1) * S
                        nc.tensor.matmul(
                            psums[ck][:],
                            v[:, a:b, :].rearrange("p r s -> p (r s)"),
                            g[:, a:b, :].rearrange("p r j -> p (r j)"),
                            start=first, stop=last,
                        )

            out_sb = singles.tile([3 * S, NCHUNK * S * NB], f32)
            if do_vec and do_pe:
                for ck in range(NCHUNK):
                    nc.vector.tensor_copy(
                        out=out_sb[:, ck * S * NB:(ck + 1) * S * NB],
                        in_=psums[ck][:],
                    )
            else:
                nc.vector.memset(out_sb[:], 0.0)
            nc.sync.dma_start(out=stats_d[:], in_=out_sb[:])

    nc.finalize()
    return nc


def _get_nc():
    if "nc" not in _CACHE:
        _CACHE["nc"] = _build_nc()
    return _CACHE["nc"]


def _prep_inputs(logits, labels):
    """Shard + host-side prep. Returns in_maps for run_bass_kernel_spmd."""
    logits = np.asarray(logits)
    labels = np.asarray(labels)
    assert logits.shape == (N, C) and logits.dtype == np.float32

    bounds = np.linspace(0.0, 1.0, 16, dtype=np.float32)[:NB]
    chosen = np.take_along_axis(
        logits, labels.reshape(-1, 1).astype(np.int64), axis=1
    ).reshape(-1)

    in_maps = []
    for c in range(NCORES):
        lo = c * ROWS_CORE
        hi = lo + ROWS_CORE
        if hi <= N:
            lg = logits[lo:hi]           # view, no copy
            ch = chosen[lo:hi]
        else:
            npad = hi - N
            lg = np.vstack([logits[lo:N], np.full((npad, C), -1.0, np.float32)])
            ch = np.concatenate([chosen[lo:N], np.zeros(npad, np.float32)])
        ch_t = np.ascontiguousarray(
            ch.reshape(T, P, R).transpose(1, 0, 2).reshape(P, T * R)
        )
        in_maps.append({"logits": lg, "chosen": ch_t, "bounds": bounds.reshape(1, NB)})
    return in_maps


def _finish(outs):
    """Fold per-core [3S, NCHUNK*S*NB] stats into (ece, acc)."""
    cum_conf = np.zeros(NB, np.float64)
    cum_acc = np.zeros(NB, np.float64)
    cum_cnt = np.zeros(NB, np.float64)
    r_idx = np.arange(S)
    for o in outs:
        o = np.asarray(o, np.float64)
        for ck in range(NCHUNK):
            blk = o[:, ck * S * NB:(ck + 1) * S * NB]
            blk = blk.reshape(S, 3, S, NB)       # [r][s][r2][j]
            cum_conf += blk[r_idx, 0, r_idx, :].sum(axis=0)
            cum_acc += blk[r_idx, 1, r_idx, :].sum(axis=0)
            cum_cnt += blk[r_idx, 2, r_idx, :].sum(axis=0)

    count = cum_cnt - np.append(cum_cnt[1:], 0.0)
    sconf = cum_conf - np.append(cum_conf[1:], 0.0)
    sacc = cum_acc - np.append(cum_acc[1:], 0.0)

    safe = count > 0
    denom = np.where(safe, count, 1.0)
    conf_in = sconf / denom
    acc_in = sacc / denom
    prop = count / float(N)
    ece = float(np.where(safe, np.abs(conf_in - acc_in) * prop, 0.0).sum() * 100.0)
    acc = float(np.where(safe, acc_in * prop, 0.0).sum() * 100.0)
    return (
        np.array([ece], np.float32),
        np.array([acc], np.float32),
    )


def _run(logits, labels, trace=False):
    from concourse.bass_utils import run_bass_kernel_spmd

    nc = _get_nc()
    in_maps = _prep_inputs(logits, labels)
    res = run_bass_kernel_spmd(
        nc, in_maps, core_ids=list(range(NCORES)), trace=trace,
    )
    outs = [r["stats"] for r in res.results]
    return _finish(outs), res


def kernel(logits, labels):
    out, _ = _run(logits, labels)
    return out

